# revision 55
# baseline (speedup 1.0000x reference)
"""Multi-head attention (N=4096, D=1024, 16 heads) on 8 trn2 NeuronCores.

Sharding: tensor-parallel over heads. Each core owns 2 heads (128 of the
1024 QKV projection columns / W_o rows), computes its heads' attention
fully on-device, applies its W_o row-slice, and returns a partial
[4096, 1024] output. The host sums the 8 partials (the "all-reduce").

Device kernel per core (all matmuls bf16, accumulation fp32 in PSUM):
  1. QT/KT/VT = (W^T x^T) chunks [128=2*64 head dims, 512 rows]; V is
     additionally PE-transposed to row-major [row 128, V0 | 1 | V1 | 1].
  2. Software-pipelined attention over (q-chunk, head, key-tile) steps:
     scores^T = K_h-slice^T Q_h-chunk -> PSUM [128, 1024];
     P = exp(scores/8) via ScalarE, PSUM -> SBUF bf16;
     [u; r]^T += (V_h | 1)^T P^T accumulated in PSUM [65, 1024].
     The V matmuls of step t-1 are emitted after scores/exp of step t so
     the in-order PE queue never delays the next exp. ScalarE (exp) is
     the bottleneck engine; everything else hides behind it.
  3. At a head seam only a single DVE copy drains PSUM (u and r, bf16);
     normalization (broadcast r via K=1 matmul, 64-lane reciprocal,
     in-place multiply) is dripped into later steps when inputs are
     long-ready, as is the final projection u^T W_o with its DMA out.
"""

import numpy as np
import ml_dtypes

import concourse.bass as bass
import concourse.tile as tile
from concourse import bacc, mybir
from concourse.bass_utils import run_bass_kernel_spmd

BF16 = mybir.dt.bfloat16
F32 = mybir.dt.float32
EXP = mybir.ActivationFunctionType.Exp

N = 4096
DIN = 1024
DOUT = 1024
NCORES = 8
DPC = 128  # dims per core = 2 heads * 64
HD = 64


def emit(tc, outs, ins, n, din):
    nc = tc.nc
    qT, kT, vT, wq, wk, wv, wo, ident = ins
    out = outs[0]

    nkt = din // 128          # contraction tiles for projections
    nch = n // 512            # 512-wide column chunks of QT/KT/VT
    njt = n // 128            # key row tiles
    is_chunk = min(1024, n)   # q rows per attention sweep
    nis = n // is_chunk
    n_half = is_chunk // 512

    import contextlib
    with contextlib.ExitStack() as ctx:
        singles = ctx.enter_context(tc.tile_pool(name="singles", bufs=1))
        qk_stream = ctx.enter_context(tc.tile_pool(name="qk_stream", bufs=18))
        vt_pool = ctx.enter_context(tc.tile_pool(name="vt_pool", bufs=3))
        pt_pool = ctx.enter_context(tc.tile_pool(name="pt_pool", bufs=5))
        ostage = ctx.enter_context(tc.tile_pool(name="ostage", bufs=8))
        u_pool = ctx.enter_context(tc.tile_pool(name="u_pool", bufs=2))
        nrm_pool = ctx.enter_context(tc.tile_pool(name="nrm_pool", bufs=2))
        ps_scores = ctx.enter_context(
            tc.tile_pool(name="ps_scores", bufs=2, space="PSUM"))
        ps_acc = ctx.enter_context(
            tc.tile_pool(name="ps_acc", bufs=1, space="PSUM"))
        ps_small = ctx.enter_context(
            tc.tile_pool(name="ps_small", bufs=2, space="PSUM"))

        # ---- weights to SBUF (identity first: it gates the PE warm-up) ----
        ident_sb = singles.tile([128, 128], BF16, tag="ident")
        nc.sync.dma_start(out=ident_sb, in_=ident)
        wq_sb = singles.tile([128, nkt, 128], BF16, tag="wq")
        wk_sb = singles.tile([128, nkt, 128], BF16, tag="wk")
        wv_sb = singles.tile([128, nkt, 128], BF16, tag="wv")
        for w_sb, w in ((wq_sb, wq), (wk_sb, wk), (wv_sb, wv)):
            nc.sync.dma_start(out=w_sb,
                              in_=w.rearrange("(kt p) c -> p kt c", p=128))
        wo0_sb = singles.tile([64, DOUT], BF16, tag="wo0")
        wo1_sb = singles.tile([64, DOUT], BF16, tag="wo1")
        nc.sync.dma_start(out=wo0_sb, in_=wo[0:64, :])
        nc.sync.dma_start(out=wo1_sb, in_=wo[64:128, :])
        # ones row at partition 64 (for the K=1 broadcast matmul)
        ones_sb = singles.tile([65, 64], BF16, tag="ones")
        nc.vector.memset(ones_sb[64:65, :], 1.0)

        # ---- PE warm-up: ~4us of junk matmuls so HAM unthrottles before
        # the projection burst (they only depend on the identity DMA) ----
        junk = ps_small.tile([128, 128], F32, tag="w", name="junk")
        for _ in range(36):
            nc.tensor.matmul(junk, lhsT=ident_sb, rhs=ident_sb,
                             start=True, stop=True)

        # ---- projection micro-unit generators ----
        qt_pairs = [None] * nis
        kt_tiles = [None] * nch
        v_tiles = [None] * njt

        def qk_chunk(src, w_sb, store, tagp, i, pair_of=None):
            """Project one 512-column chunk of QT/KT/VT; yields micro-units.

            With pair_of=(tiles, pair_idx, half), the result lands in half of
            a persistent [128, is_chunk] tile so consumers get one wide AP.
            """
            box = []
            for kt in range(nkt):
                def unit(kt=kt):
                    if kt == 0:
                        box.append(ps_small.tile([128, 512], F32, tag="w",
                                                 name=f"ps_{tagp}{i}"))
                    st = qk_stream.tile([128, 512], BF16, tag="qkst",
                                        name=f"st_{tagp}{i}_{kt}")
                    dma_eng = nc.sync if kt % 2 == 0 else nc.gpsimd
                    dma_eng.dma_start(
                        out=st,
                        in_=src[kt * 128:(kt + 1) * 128, i * 512:(i + 1) * 512])
                    nc.tensor.matmul(box[0], lhsT=w_sb[:, kt, :], rhs=st,
                                     start=(kt == 0), stop=(kt == nkt - 1))
                yield unit
            def fin():
                if pair_of is not None:
                    tiles, pi, half = pair_of
                    if tiles[pi] is None:
                        tiles[pi] = singles.tile([128, is_chunk], BF16,
                                                 tag=f"{tagp}p{pi}",
                                                 name=f"{tagp}p{pi}")
                    nc.vector.tensor_copy(
                        tiles[pi][:, half * 512:(half + 1) * 512], box[0])
                    return
                pool = singles if store is not None else vt_pool
                dst = pool.tile([128, 512], BF16, tag=f"{tagp}{i}" if store is not None else "vtc",
                                name=f"{tagp}{i}")
                nc.vector.tensor_copy(dst, box[0])
                if store is not None:
                    store[i] = dst
                else:
                    box.append(dst)
            yield fin
            if store is None and pair_of is None:
                # V: transpose each 128-row block to row-major V0 | 1 | V1 | 1
                for r in range(4):
                    def tunit(r=r):
                        jt = 4 * i + r
                        vtc = box[1]
                        tp = ps_small.tile([128, 128], BF16, tag="w",
                                           name=f"tp{jt}")
                        nc.tensor.transpose(tp, vtc[:, r * 128:(r + 1) * 128],
                                            ident_sb)
                        v_t = singles.tile([128, 130], BF16, tag=f"v{jt}",
                                           name=f"v{jt}")
                        nc.vector.tensor_copy(v_t[:, 0:64], tp[:, 0:64])
                        nc.vector.tensor_copy(v_t[:, 65:129], tp[:, 64:128])
                        nc.vector.memset(v_t[:, 64:65], 1.0)
                        nc.vector.memset(v_t[:, 129:130], 1.0)
                        v_tiles[jt] = v_t
                    yield tunit

        # up-front: only what step 0 needs (K0, V0, Q0-1); the rest drips
        # into the attention loop ahead of its first use.
        up_q = min(2, nch)
        for u_ in qk_chunk(kT, wk_sb, kt_tiles, "kt", 0):
            u_()
        for u_ in qk_chunk(vT, wv_sb, None, "vt", 0):
            u_()
        def q_chunk(i):
            return qk_chunk(qT, wq_sb, None, "qt", i,
                            pair_of=(qt_pairs, i // n_half, i % n_half))
        for i in range(up_q):
            for u_ in q_chunk(i):
                u_()
        drip = []
        for i in range(1, nch):
            drip.extend(qk_chunk(kT, wk_sb, kt_tiles, "kt", i))
            drip.extend(qk_chunk(vT, wv_sb, None, "vt", i))
        for i in range(up_q, nch):
            drip.extend(q_chunk(i))
        drip.reverse()  # pop() from the end

        # ---- deferred epilogue units ----
        def norm_unit(uraw, half, tag):
            def unit():
                sl = slice(half * 512, (half + 1) * 512)
                bc_ps = ps_small.tile([64, 512], F32, tag="w",
                                      name=f"bc{tag}_{half}")
                nc.tensor.matmul(bc_ps, lhsT=ones_sb[64:65, :],
                                 rhs=uraw[64:65, sl], start=True, stop=True)
                rbc = nrm_pool.tile([64, 512], F32, tag="rbc",
                                    name=f"rbc{tag}_{half}")
                nc.vector.reciprocal(rbc, bc_ps)
                nc.vector.tensor_mul(uraw[0:64, sl], uraw[0:64, sl], rbc)
            return unit

        def out_unit(us, isup, it, wc, eng="v"):
            def unit():
                row0 = isup * is_chunk + it * 128
                po = ps_small.tile([128, 512], F32, tag="w",
                                   name=f"po{isup}_{it}_{wc}")
                nc.tensor.matmul(po, lhsT=us[0][0:64, it * 128:(it + 1) * 128],
                                 rhs=wo0_sb[:, wc * 512:(wc + 1) * 512],
                                 start=True, stop=False)
                nc.tensor.matmul(po, lhsT=us[1][0:64, it * 128:(it + 1) * 128],
                                 rhs=wo1_sb[:, wc * 512:(wc + 1) * 512],
                                 start=False, stop=True)
                ot = ostage.tile([128, 512], F32, tag="ot",
                                 name=f"ot{isup}_{it}_{wc}")
                if eng == "v":
                    nc.vector.tensor_copy(ot, po)
                else:
                    nc.scalar.copy(ot, po)
                dma_eng = nc.gpsimd if (it + wc) % 2 == 0 else nc.sync
                dma_eng.dma_start(
                    out=out[row0:row0 + 128, wc * 512:(wc + 1) * 512],
                    in_=ot)
            return unit

        # ---- software-pipelined attention (V matmuls 2 steps behind) ----
        steps = [(isup, h, jt)
                 for isup in range(nis) for h in range(2) for jt in range(njt)]
        pending_v = []   # up to 2 entries
        drains = []      # (due_step, fn): PSUM accumulator drain copies
        epi = []         # (due_step, fn): normalize + out-projection units
        us = None
        accs = {}

        def flush_v():
            acc_p, vslice_p, pt_p, last_p = pending_v.pop(0)
            for half in range(n_half):
                nc.tensor.matmul(
                    acc_p[:, half * 512:(half + 1) * 512],
                    lhsT=vslice_p,
                    rhs=pt_p[:, half * 512:(half + 1) * 512],
                    start=(last_p == 0), stop=(last_p == njt - 1))

        for t, (isup, h, jt) in enumerate(steps):
            if jt == 0:
                if h == 0:
                    us = [None, None]
                accs[(isup, h)] = ps_acc.tile([65, is_chunk], F32, tag="acc",
                                              name=f"acc{isup}_{h}")
            for _ in range(6 if t < 32 else (4 if t < 48 else 2)):
                if drip:
                    drip.pop()()
            # scores + exp for step t
            sc = ps_scores.tile([128, is_chunk], F32, tag="s",
                                name=f"sc{isup}_{h}_{jt}")
            ktile = kt_tiles[jt // 4]
            for half in range(n_half):
                nc.tensor.matmul(
                    sc[:, half * 512:(half + 1) * 512],
                    lhsT=ktile[h * 64:(h + 1) * 64,
                               (jt % 4) * 128:(jt % 4) * 128 + 128],
                    rhs=qt_pairs[isup][h * 64:(h + 1) * 64,
                                       half * 512:(half + 1) * 512],
                    start=True, stop=True)
            pt = pt_pool.tile([128, is_chunk], BF16, tag="pt",
                              name=f"pt{isup}_{h}_{jt}")
            nc.scalar.activation(pt, sc, EXP, scale=0.125)
            # V matmuls of step t-3
            if len(pending_v) == 3:
                flush_v()
            acc = accs[(isup, h)]
            vlo = 0 if h == 0 else 65
            pending_v.append((acc, v_tiles[jt][:, vlo:vlo + 65], pt, jt))
            # deferred work whose inputs are long ready
            while drains and drains[0][0] <= t:
                drains.pop(0)[1]()
            if t % 2 == 0 and epi and epi[0][0] <= t:
                epi.pop(0)[1]()
            if jt == njt - 1:
                # head seam: drain the PSUM accumulator once its final V
                # matmuls (emitted at step t+3) are in; normalize later.
                uraw = u_pool.tile([65, is_chunk], BF16, tag=f"u{h}",
                                   name=f"uraw{isup}_{h}")
                us[h] = uraw
                def drain(acc=acc, uraw=uraw):
                    nc.vector.tensor_copy(uraw, acc)
                    # filler matmuls: keep the PE busy across the seam's DVE
                    # latency so HAM never clock-gates the array
                    for _ in range(6):
                        nc.tensor.matmul(junk, lhsT=ident_sb, rhs=ident_sb,
                                         start=True, stop=True)
                drains.append((t + 3, drain))
                if t + 1 < len(steps):
                    epi.extend((t + 4, norm_unit(uraw, half, f"{isup}_{h}"))
                               for half in range(n_half))
                else:
                    tail_norm = [norm_unit(uraw, half, f"{isup}_{h}")
                                 for half in range(n_half)]
                if h == 1 and isup < nis - 1:
                    epi.extend((t + 5, out_unit(us, isup, it, wc))
                               for it in range(is_chunk // 128)
                               for wc in range(DOUT // 512))

        # ---- tail: last head's V matmuls, drain, normalize + out-proj ----
        while pending_v:
            flush_v()
        while drip:
            drip.pop()()
        for _, fn in drains:
            fn()
        # keep the PE warm across the DVE drain/normalize latency
        for _ in range(14):
            nc.tensor.matmul(junk, lhsT=ident_sb, rhs=ident_sb,
                             start=True, stop=True)
        for _, fn in epi:
            fn()
        isup = nis - 1
        nh = is_chunk // 128
        for half in range(n_half):
            tail_norm[half]()
            for it in range(half * nh // n_half, (half + 1) * nh // n_half):
                for wc in range(DOUT // 512):
                    out_unit(us, isup, it, wc,
                             eng=("s" if (it + wc) % 2 else "v"))()


def build(n=N, din=DIN):
    nc = bacc.Bacc("TRN2", target_bir_lowering=False, debug=False,
                   num_devices=NCORES)
    qT = nc.dram_tensor("qT", [din, n], BF16, kind="ExternalInput").ap()
    kT = nc.dram_tensor("kT", [din, n], BF16, kind="ExternalInput").ap()
    vT = nc.dram_tensor("vT", [din, n], BF16, kind="ExternalInput").ap()
    wq = nc.dram_tensor("wq", [din, DPC], BF16, kind="ExternalInput").ap()
    wk = nc.dram_tensor("wk", [din, DPC], BF16, kind="ExternalInput").ap()
    wv = nc.dram_tensor("wv", [din, DPC], BF16, kind="ExternalInput").ap()
    wo = nc.dram_tensor("wo", [DPC, DOUT], BF16, kind="ExternalInput").ap()
    ident = nc.dram_tensor("ident", [128, 128], BF16, kind="ExternalInput").ap()
    out = nc.dram_tensor("out", [n, DOUT], F32, kind="ExternalOutput").ap()
    with tile.TileContext(nc) as tc:
        emit(tc, [out], [qT, kT, vT, wq, wk, wv, wo, ident], n, din)
    nc.compile()
    return nc


_NC_CACHE = {}


def _get_nc(n=N, din=DIN):
    key = (n, din)
    if key not in _NC_CACHE:
        _NC_CACHE[key] = build(n, din)
    return _NC_CACHE[key]


def make_in_maps(q, k, v, W_q, W_k, W_v, W_o):
    bf = ml_dtypes.bfloat16
    qT = np.ascontiguousarray(np.asarray(q, dtype=np.float32).T).astype(bf)
    kT = np.ascontiguousarray(np.asarray(k, dtype=np.float32).T).astype(bf)
    vT = np.ascontiguousarray(np.asarray(v, dtype=np.float32).T).astype(bf)
    W_q = np.asarray(W_q, dtype=np.float32)
    W_k = np.asarray(W_k, dtype=np.float32)
    W_v = np.asarray(W_v, dtype=np.float32)
    W_o = np.asarray(W_o, dtype=np.float32)
    ident = np.eye(128, dtype=bf)
    in_maps = []
    for c in range(NCORES):
        sl = slice(DPC * c, DPC * (c + 1))
        in_maps.append({
            "qT": qT, "kT": kT, "vT": vT,
            "wq": np.ascontiguousarray(W_q[:, sl]).astype(bf),
            "wk": np.ascontiguousarray(W_k[:, sl]).astype(bf),
            "wv": np.ascontiguousarray(W_v[:, sl]).astype(bf),
            "wo": np.ascontiguousarray(W_o[sl, :]).astype(bf),
            "ident": ident,
        })
    return in_maps


def run(q, k, v, W_q, W_k, W_v, W_o, trace=False):
    n = q.shape[0]
    nc = _get_nc(n=n, din=q.shape[1])
    in_maps = make_in_maps(q, k, v, W_q, W_k, W_v, W_o)
    res = run_bass_kernel_spmd(nc, in_maps, list(range(NCORES)), trace=trace)
    out = res.results[0]["out"].astype(np.float32)
    for c in range(1, NCORES):
        out += res.results[c]["out"]
    return out, res


def kernel(q, k, v, W_q, W_k, W_v, W_o):
    out, _ = run(q, k, v, W_q, W_k, W_v, W_o)
    return out


# revision 56
# speedup vs baseline: 1.0176x; 1.0176x over previous
"""Multi-head attention (N=4096, D=1024, 16 heads) on 8 trn2 NeuronCores.

Sharding: tensor-parallel over heads. Each core owns 2 heads (128 of the
1024 QKV projection columns / W_o rows), computes its heads' attention
fully on-device, applies its W_o row-slice, and returns a partial
[4096, 1024] output. The host sums the 8 partials (the "all-reduce").

Device kernel per core (all matmuls bf16, accumulation fp32 in PSUM):
  1. QT/KT/VT = (W^T x^T) chunks [128=2*64 head dims, 512 rows]; V is
     additionally PE-transposed to row-major [row 128, V0 | 1 | V1 | 1].
  2. Software-pipelined attention over (q-chunk, head, key-tile) steps:
     scores^T = K_h-slice^T Q_h-chunk -> PSUM [128, 1024];
     P = exp(scores/8) via ScalarE, PSUM -> SBUF bf16;
     [u; r]^T += (V_h | 1)^T P^T accumulated in PSUM [65, 1024].
     The V matmuls of step t-1 are emitted after scores/exp of step t so
     the in-order PE queue never delays the next exp. ScalarE (exp) is
     the bottleneck engine; everything else hides behind it.
  3. At a head seam only a single DVE copy drains PSUM (u and r, bf16);
     normalization (broadcast r via K=1 matmul, 64-lane reciprocal,
     in-place multiply) is dripped into later steps when inputs are
     long-ready, as is the final projection u^T W_o with its DMA out.
"""

import numpy as np
import ml_dtypes

import concourse.bass as bass
import concourse.tile as tile
from concourse import bacc, mybir
from concourse.bass_utils import run_bass_kernel_spmd

BF16 = mybir.dt.bfloat16
F32 = mybir.dt.float32
EXP = mybir.ActivationFunctionType.Exp

N = 4096
DIN = 1024
DOUT = 1024
NCORES = 8
DPC = 128  # dims per core = 2 heads * 64
HD = 64


def emit(tc, outs, ins, n, din):
    nc = tc.nc
    qT, kT, vT, wq, wk, wv, wo, ident = ins
    out = outs[0]

    nkt = din // 128          # contraction tiles for projections
    nch = n // 512            # 512-wide column chunks of QT/KT/VT
    njt = n // 128            # key row tiles
    is_chunk = min(1024, n)   # q rows per attention sweep
    nis = n // is_chunk
    n_half = is_chunk // 512

    import contextlib
    with contextlib.ExitStack() as ctx:
        singles = ctx.enter_context(tc.tile_pool(name="singles", bufs=1))
        qk_stream = ctx.enter_context(tc.tile_pool(name="qk_stream", bufs=18))
        vt_pool = ctx.enter_context(tc.tile_pool(name="vt_pool", bufs=3))
        pt_pool = ctx.enter_context(tc.tile_pool(name="pt_pool", bufs=5))
        ostage = ctx.enter_context(tc.tile_pool(name="ostage", bufs=8))
        u_pool = ctx.enter_context(tc.tile_pool(name="u_pool", bufs=2))
        nrm_pool = ctx.enter_context(tc.tile_pool(name="nrm_pool", bufs=2))
        ps_scores = ctx.enter_context(
            tc.tile_pool(name="ps_scores", bufs=2, space="PSUM"))
        ps_acc = ctx.enter_context(
            tc.tile_pool(name="ps_acc", bufs=1, space="PSUM"))
        ps_small = ctx.enter_context(
            tc.tile_pool(name="ps_small", bufs=2, space="PSUM"))

        # ---- weights to SBUF (identity first: it gates the PE warm-up) ----
        ident_sb = singles.tile([128, 128], BF16, tag="ident")
        nc.sync.dma_start(out=ident_sb, in_=ident)
        wq_sb = singles.tile([128, nkt, 128], BF16, tag="wq")
        wk_sb = singles.tile([128, nkt, 128], BF16, tag="wk")
        wv_sb = singles.tile([128, nkt, 128], BF16, tag="wv")
        for w_sb, w in ((wq_sb, wq), (wk_sb, wk), (wv_sb, wv)):
            nc.sync.dma_start(out=w_sb,
                              in_=w.rearrange("(kt p) c -> p kt c", p=128))
        wo0_sb = singles.tile([64, DOUT], BF16, tag="wo0")
        wo1_sb = singles.tile([64, DOUT], BF16, tag="wo1")
        nc.sync.dma_start(out=wo0_sb, in_=wo[0:64, :])
        nc.sync.dma_start(out=wo1_sb, in_=wo[64:128, :])
        # ones row at partition 64 (for the K=1 broadcast matmul)
        ones_sb = singles.tile([65, 64], BF16, tag="ones")
        nc.vector.memset(ones_sb[64:65, :], 1.0)

        # ---- PE warm-up: ~4us of junk matmuls so HAM unthrottles before
        # the projection burst (they only depend on the identity DMA) ----
        junk = ps_small.tile([128, 128], F32, tag="w", name="junk")
        for _ in range(36):
            nc.tensor.matmul(junk, lhsT=ident_sb, rhs=ident_sb,
                             start=True, stop=True)

        # ---- projection micro-unit generators ----
        qt_pairs = [None] * nis
        kt_tiles = [None] * nch
        v_tiles = [None] * njt

        def qk_chunk(src, w_sb, store, tagp, i, pair_of=None):
            """Project one 512-column chunk of QT/KT/VT; yields micro-units.

            With pair_of=(tiles, pair_idx, half), the result lands in half of
            a persistent [128, is_chunk] tile so consumers get one wide AP.
            """
            box = []
            for kt in range(nkt):
                def unit(kt=kt):
                    if kt == 0:
                        box.append(ps_small.tile([128, 512], F32, tag="w",
                                                 name=f"ps_{tagp}{i}"))
                    st = qk_stream.tile([128, 512], BF16, tag="qkst",
                                        name=f"st_{tagp}{i}_{kt}")
                    dma_eng = nc.sync if kt % 2 == 0 else nc.gpsimd
                    dma_eng.dma_start(
                        out=st,
                        in_=src[kt * 128:(kt + 1) * 128, i * 512:(i + 1) * 512])
                    nc.tensor.matmul(box[0], lhsT=w_sb[:, kt, :], rhs=st,
                                     start=(kt == 0), stop=(kt == nkt - 1))
                yield unit
            def fin():
                if pair_of is not None:
                    tiles, pi, half = pair_of
                    if tiles[pi] is None:
                        tiles[pi] = singles.tile([128, is_chunk], BF16,
                                                 tag=f"{tagp}p{pi}",
                                                 name=f"{tagp}p{pi}")
                    nc.vector.tensor_copy(
                        tiles[pi][:, half * 512:(half + 1) * 512], box[0])
                    return
                pool = singles if store is not None else vt_pool
                dst = pool.tile([128, 512], BF16, tag=f"{tagp}{i}" if store is not None else "vtc",
                                name=f"{tagp}{i}")
                nc.vector.tensor_copy(dst, box[0])
                if store is not None:
                    store[i] = dst
                else:
                    box.append(dst)
            yield fin
            if store is None and pair_of is None:
                # V: transpose each 128-row block to row-major V0 | 1 | V1 | 1
                for r in range(4):
                    def tunit(r=r):
                        jt = 4 * i + r
                        vtc = box[1]
                        tp = ps_small.tile([128, 128], BF16, tag="w",
                                           name=f"tp{jt}")
                        nc.tensor.transpose(tp, vtc[:, r * 128:(r + 1) * 128],
                                            ident_sb)
                        v_t = singles.tile([128, 130], BF16, tag=f"v{jt}",
                                           name=f"v{jt}")
                        nc.vector.tensor_copy(v_t[:, 0:64], tp[:, 0:64])
                        nc.vector.tensor_copy(v_t[:, 65:129], tp[:, 64:128])
                        nc.vector.memset(v_t[:, 64:65], 1.0)
                        nc.vector.memset(v_t[:, 129:130], 1.0)
                        v_tiles[jt] = v_t
                    yield tunit

        # up-front: only what step 0 needs (K0, V0, Q0-1); the rest drips
        # into the attention loop ahead of its first use.
        up_q = min(2, nch)
        for u_ in qk_chunk(kT, wk_sb, kt_tiles, "kt", 0):
            u_()
        for u_ in qk_chunk(vT, wv_sb, None, "vt", 0):
            u_()
        def q_chunk(i):
            return qk_chunk(qT, wq_sb, None, "qt", i,
                            pair_of=(qt_pairs, i // n_half, i % n_half))
        for i in range(up_q):
            for u_ in q_chunk(i):
                u_()
        drip = []
        for i in range(1, nch):
            drip.extend(qk_chunk(kT, wk_sb, kt_tiles, "kt", i))
            drip.extend(qk_chunk(vT, wv_sb, None, "vt", i))
        for i in range(up_q, nch):
            drip.extend(q_chunk(i))
        drip.reverse()  # pop() from the end

        # ---- deferred epilogue units ----
        def norm_unit(uraw, half, tag):
            def unit():
                sl = slice(half * 512, (half + 1) * 512)
                bc_ps = ps_small.tile([64, 512], F32, tag="w",
                                      name=f"bc{tag}_{half}")
                nc.tensor.matmul(bc_ps, lhsT=ones_sb[64:65, :],
                                 rhs=uraw[64:65, sl], start=True, stop=True)
                rbc = nrm_pool.tile([64, 512], F32, tag="rbc",
                                    name=f"rbc{tag}_{half}")
                nc.vector.reciprocal_approx_fast(out=rbc, in_=bc_ps)
                nc.vector.tensor_mul(uraw[0:64, sl], uraw[0:64, sl], rbc)
            return unit

        def out_unit(us, isup, it, wc, eng="v"):
            def unit():
                row0 = isup * is_chunk + it * 128
                po = ps_small.tile([128, 512], F32, tag="w",
                                   name=f"po{isup}_{it}_{wc}")
                nc.tensor.matmul(po, lhsT=us[0][0:64, it * 128:(it + 1) * 128],
                                 rhs=wo0_sb[:, wc * 512:(wc + 1) * 512],
                                 start=True, stop=False)
                nc.tensor.matmul(po, lhsT=us[1][0:64, it * 128:(it + 1) * 128],
                                 rhs=wo1_sb[:, wc * 512:(wc + 1) * 512],
                                 start=False, stop=True)
                ot = ostage.tile([128, 512], F32, tag="ot",
                                 name=f"ot{isup}_{it}_{wc}")
                if eng == "v":
                    nc.vector.tensor_copy(ot, po)
                else:
                    nc.scalar.copy(ot, po)
                dma_eng = nc.gpsimd if (it + wc) % 2 == 0 else nc.sync
                dma_eng.dma_start(
                    out=out[row0:row0 + 128, wc * 512:(wc + 1) * 512],
                    in_=ot)
            return unit

        # ---- software-pipelined attention (V matmuls 2 steps behind) ----
        steps = [(isup, h, jt)
                 for isup in range(nis) for h in range(2) for jt in range(njt)]
        pending_v = []   # up to 2 entries
        drains = []      # (due_step, fn): PSUM accumulator drain copies
        epi = []         # (due_step, fn): normalize + out-projection units
        us = None
        accs = {}

        def flush_v():
            acc_p, vslice_p, pt_p, last_p = pending_v.pop(0)
            for half in range(n_half):
                nc.tensor.matmul(
                    acc_p[:, half * 512:(half + 1) * 512],
                    lhsT=vslice_p,
                    rhs=pt_p[:, half * 512:(half + 1) * 512],
                    start=(last_p == 0), stop=(last_p == njt - 1))

        for t, (isup, h, jt) in enumerate(steps):
            if jt == 0:
                if h == 0:
                    us = [None, None]
                accs[(isup, h)] = ps_acc.tile([65, is_chunk], F32, tag="acc",
                                              name=f"acc{isup}_{h}")
            for _ in range(6 if t < 32 else (4 if t < 48 else 2)):
                if drip:
                    drip.pop()()
            # scores + exp for step t
            sc = ps_scores.tile([128, is_chunk], F32, tag="s",
                                name=f"sc{isup}_{h}_{jt}")
            ktile = kt_tiles[jt // 4]
            for half in range(n_half):
                nc.tensor.matmul(
                    sc[:, half * 512:(half + 1) * 512],
                    lhsT=ktile[h * 64:(h + 1) * 64,
                               (jt % 4) * 128:(jt % 4) * 128 + 128],
                    rhs=qt_pairs[isup][h * 64:(h + 1) * 64,
                                       half * 512:(half + 1) * 512],
                    start=True, stop=True)
            pt = pt_pool.tile([128, is_chunk], BF16, tag="pt",
                              name=f"pt{isup}_{h}_{jt}")
            nc.scalar.activation(pt, sc, EXP, scale=0.125)
            # V matmuls of step t-3
            if len(pending_v) == 3:
                flush_v()
            acc = accs[(isup, h)]
            vlo = 0 if h == 0 else 65
            pending_v.append((acc, v_tiles[jt][:, vlo:vlo + 65], pt, jt))
            # deferred work whose inputs are long ready
            while drains and drains[0][0] <= t:
                drains.pop(0)[1]()
            if t % 2 == 0 and epi and epi[0][0] <= t:
                epi.pop(0)[1]()
            if jt == njt - 1:
                # head seam: drain the PSUM accumulator once its final V
                # matmuls (emitted at step t+3) are in; normalize later.
                uraw = u_pool.tile([65, is_chunk], BF16, tag=f"u{h}",
                                   name=f"uraw{isup}_{h}")
                us[h] = uraw
                def drain(acc=acc, uraw=uraw):
                    nc.vector.tensor_copy(uraw, acc)
                    # filler matmuls: keep the PE busy across the seam's DVE
                    # latency so HAM never clock-gates the array
                    for _ in range(6):
                        nc.tensor.matmul(junk, lhsT=ident_sb, rhs=ident_sb,
                                         start=True, stop=True)
                drains.append((t + 3, drain))
                if t + 1 < len(steps):
                    epi.extend((t + 4, norm_unit(uraw, half, f"{isup}_{h}"))
                               for half in range(n_half))
                else:
                    tail_norm = [norm_unit(uraw, half, f"{isup}_{h}")
                                 for half in range(n_half)]
                if h == 1 and isup < nis - 1:
                    epi.extend((t + 5, out_unit(us, isup, it, wc))
                               for it in range(is_chunk // 128)
                               for wc in range(DOUT // 512))

        # ---- tail: last head's V matmuls, drain, normalize + out-proj ----
        while pending_v:
            flush_v()
        while drip:
            drip.pop()()
        for _, fn in drains:
            fn()
        # keep the PE warm across the DVE drain/normalize latency
        for _ in range(14):
            nc.tensor.matmul(junk, lhsT=ident_sb, rhs=ident_sb,
                             start=True, stop=True)
        for _, fn in epi:
            fn()
        isup = nis - 1
        nh = is_chunk // 128
        for half in range(n_half):
            tail_norm[half]()
            for it in range(half * nh // n_half, (half + 1) * nh // n_half):
                for wc in range(DOUT // 512):
                    out_unit(us, isup, it, wc,
                             eng=("s" if (it + wc) % 2 else "v"))()


def build(n=N, din=DIN):
    nc = bacc.Bacc("TRN2", target_bir_lowering=False, debug=False,
                   num_devices=NCORES)
    qT = nc.dram_tensor("qT", [din, n], BF16, kind="ExternalInput").ap()
    kT = nc.dram_tensor("kT", [din, n], BF16, kind="ExternalInput").ap()
    vT = nc.dram_tensor("vT", [din, n], BF16, kind="ExternalInput").ap()
    wq = nc.dram_tensor("wq", [din, DPC], BF16, kind="ExternalInput").ap()
    wk = nc.dram_tensor("wk", [din, DPC], BF16, kind="ExternalInput").ap()
    wv = nc.dram_tensor("wv", [din, DPC], BF16, kind="ExternalInput").ap()
    wo = nc.dram_tensor("wo", [DPC, DOUT], BF16, kind="ExternalInput").ap()
    ident = nc.dram_tensor("ident", [128, 128], BF16, kind="ExternalInput").ap()
    out = nc.dram_tensor("out", [n, DOUT], F32, kind="ExternalOutput").ap()
    with tile.TileContext(nc) as tc:
        emit(tc, [out], [qT, kT, vT, wq, wk, wv, wo, ident], n, din)
    nc.compile()
    return nc


_NC_CACHE = {}


def _get_nc(n=N, din=DIN):
    key = (n, din)
    if key not in _NC_CACHE:
        _NC_CACHE[key] = build(n, din)
    return _NC_CACHE[key]


def make_in_maps(q, k, v, W_q, W_k, W_v, W_o):
    bf = ml_dtypes.bfloat16
    qT = np.ascontiguousarray(np.asarray(q, dtype=np.float32).T).astype(bf)
    kT = np.ascontiguousarray(np.asarray(k, dtype=np.float32).T).astype(bf)
    vT = np.ascontiguousarray(np.asarray(v, dtype=np.float32).T).astype(bf)
    W_q = np.asarray(W_q, dtype=np.float32)
    W_k = np.asarray(W_k, dtype=np.float32)
    W_v = np.asarray(W_v, dtype=np.float32)
    W_o = np.asarray(W_o, dtype=np.float32)
    ident = np.eye(128, dtype=bf)
    in_maps = []
    for c in range(NCORES):
        sl = slice(DPC * c, DPC * (c + 1))
        in_maps.append({
            "qT": qT, "kT": kT, "vT": vT,
            "wq": np.ascontiguousarray(W_q[:, sl]).astype(bf),
            "wk": np.ascontiguousarray(W_k[:, sl]).astype(bf),
            "wv": np.ascontiguousarray(W_v[:, sl]).astype(bf),
            "wo": np.ascontiguousarray(W_o[sl, :]).astype(bf),
            "ident": ident,
        })
    return in_maps


def run(q, k, v, W_q, W_k, W_v, W_o, trace=False):
    n = q.shape[0]
    nc = _get_nc(n=n, din=q.shape[1])
    in_maps = make_in_maps(q, k, v, W_q, W_k, W_v, W_o)
    res = run_bass_kernel_spmd(nc, in_maps, list(range(NCORES)), trace=trace)
    out = res.results[0]["out"].astype(np.float32)
    for c in range(1, NCORES):
        out += res.results[c]["out"]
    return out, res


def kernel(q, k, v, W_q, W_k, W_v, W_o):
    out, _ = run(q, k, v, W_q, W_k, W_v, W_o)
    return out


# revision 58
# speedup vs baseline: 1.0197x; 1.0021x over previous
"""Multi-head attention (N=4096, D=1024, 16 heads) on 8 trn2 NeuronCores.

Sharding: tensor-parallel over heads. Each core owns 2 heads (128 of the
1024 QKV projection columns / W_o rows), computes its heads' attention
fully on-device, applies its W_o row-slice, and returns a partial
[4096, 1024] output. The host sums the 8 partials (the "all-reduce").

Device kernel per core (all matmuls bf16, accumulation fp32 in PSUM):
  1. QT/KT/VT = (W^T x^T) chunks [128=2*64 head dims, 512 rows]; V is
     additionally PE-transposed to row-major [row 128, V0 | 1 | V1 | 1].
  2. Software-pipelined attention over (q-chunk, head, key-tile) steps:
     scores^T = K_h-slice^T Q_h-chunk -> PSUM [128, 1024];
     P = exp(scores/8) via ScalarE, PSUM -> SBUF bf16;
     [u; r]^T += (V_h | 1)^T P^T accumulated in PSUM [65, 1024].
     The V matmuls of step t-1 are emitted after scores/exp of step t so
     the in-order PE queue never delays the next exp. ScalarE (exp) is
     the bottleneck engine; everything else hides behind it.
  3. At a head seam only a single DVE copy drains PSUM (u and r, bf16);
     normalization (broadcast r via K=1 matmul, 64-lane reciprocal,
     in-place multiply) is dripped into later steps when inputs are
     long-ready, as is the final projection u^T W_o with its DMA out.
"""

import numpy as np
import ml_dtypes

import concourse.bass as bass
import concourse.tile as tile
from concourse import bacc, mybir
from concourse.bass_utils import run_bass_kernel_spmd

BF16 = mybir.dt.bfloat16
F32 = mybir.dt.float32
EXP = mybir.ActivationFunctionType.Exp

N = 4096
DIN = 1024
DOUT = 1024
NCORES = 8
DPC = 128  # dims per core = 2 heads * 64
HD = 64


def emit(tc, outs, ins, n, din):
    nc = tc.nc
    qT, kT, vT, wq, wk, wv, wo, ident = ins
    out = outs[0]

    nkt = din // 128          # contraction tiles for projections
    nch = n // 512            # 512-wide column chunks of QT/KT/VT
    njt = n // 128            # key row tiles
    is_chunk = min(1024, n)   # q rows per attention sweep
    nis = n // is_chunk
    n_half = is_chunk // 512

    import contextlib
    with contextlib.ExitStack() as ctx:
        singles = ctx.enter_context(tc.tile_pool(name="singles", bufs=1))
        qk_stream = ctx.enter_context(tc.tile_pool(name="qk_stream", bufs=18))
        vt_pool = ctx.enter_context(tc.tile_pool(name="vt_pool", bufs=3))
        pt_pool = ctx.enter_context(tc.tile_pool(name="pt_pool", bufs=5))
        ostage = ctx.enter_context(tc.tile_pool(name="ostage", bufs=8))
        u_pool = ctx.enter_context(tc.tile_pool(name="u_pool", bufs=2))
        nrm_pool = ctx.enter_context(tc.tile_pool(name="nrm_pool", bufs=2))
        ps_scores = ctx.enter_context(
            tc.tile_pool(name="ps_scores", bufs=2, space="PSUM"))
        ps_acc = ctx.enter_context(
            tc.tile_pool(name="ps_acc", bufs=1, space="PSUM"))
        ps_small = ctx.enter_context(
            tc.tile_pool(name="ps_small", bufs=2, space="PSUM"))

        # ---- weights to SBUF (identity first: it gates the PE warm-up) ----
        ident_sb = singles.tile([128, 128], BF16, tag="ident")
        nc.sync.dma_start(out=ident_sb, in_=ident)
        wq_sb = singles.tile([128, nkt, 128], BF16, tag="wq")
        wk_sb = singles.tile([128, nkt, 128], BF16, tag="wk")
        wv_sb = singles.tile([128, nkt, 128], BF16, tag="wv")
        for w_sb, w in ((wq_sb, wq), (wk_sb, wk), (wv_sb, wv)):
            nc.sync.dma_start(out=w_sb,
                              in_=w.rearrange("(kt p) c -> p kt c", p=128))
        wo0_sb = singles.tile([64, DOUT], BF16, tag="wo0")
        wo1_sb = singles.tile([64, DOUT], BF16, tag="wo1")
        nc.sync.dma_start(out=wo0_sb, in_=wo[0:64, :])
        nc.sync.dma_start(out=wo1_sb, in_=wo[64:128, :])
        # ones row at partition 64 (for the K=1 broadcast matmul)
        ones_sb = singles.tile([65, 64], BF16, tag="ones")
        nc.vector.memset(ones_sb[64:65, :], 1.0)

        # ---- PE warm-up: ~4us of junk matmuls so HAM unthrottles before
        # the projection burst (they only depend on the identity DMA) ----
        junk = ps_small.tile([128, 128], F32, tag="w", name="junk")
        for _ in range(36):
            nc.tensor.matmul(junk, lhsT=ident_sb, rhs=ident_sb,
                             start=True, stop=True)

        # ---- projection micro-unit generators ----
        qt_pairs = [None] * nis
        kt_tiles = [None] * nch
        v_tiles = [None] * njt

        def qk_chunk(src, w_sb, store, tagp, i, pair_of=None):
            """Project one 512-column chunk of QT/KT/VT; yields micro-units.

            With pair_of=(tiles, pair_idx, half), the result lands in half of
            a persistent [128, is_chunk] tile so consumers get one wide AP.
            """
            box = []
            for kt in range(nkt):
                def unit(kt=kt):
                    if kt == 0:
                        box.append(ps_small.tile([128, 512], F32, tag="w",
                                                 name=f"ps_{tagp}{i}"))
                    st = qk_stream.tile([128, 512], BF16, tag="qkst",
                                        name=f"st_{tagp}{i}_{kt}")
                    dma_eng = nc.sync if kt % 2 == 0 else nc.gpsimd
                    dma_eng.dma_start(
                        out=st,
                        in_=src[kt * 128:(kt + 1) * 128, i * 512:(i + 1) * 512])
                    nc.tensor.matmul(box[0], lhsT=w_sb[:, kt, :], rhs=st,
                                     start=(kt == 0), stop=(kt == nkt - 1))
                yield unit
            def fin():
                if pair_of is not None:
                    tiles, pi, half = pair_of
                    if tiles[pi] is None:
                        tiles[pi] = singles.tile([128, is_chunk], BF16,
                                                 tag=f"{tagp}p{pi}",
                                                 name=f"{tagp}p{pi}")
                    nc.vector.tensor_copy(
                        tiles[pi][:, half * 512:(half + 1) * 512], box[0])
                    return
                pool = singles if store is not None else vt_pool
                dst = pool.tile([128, 512], BF16, tag=f"{tagp}{i}" if store is not None else "vtc",
                                name=f"{tagp}{i}")
                nc.vector.tensor_copy(dst, box[0])
                if store is not None:
                    store[i] = dst
                else:
                    box.append(dst)
            yield fin
            if store is None and pair_of is None:
                # V: transpose each 128-row block to row-major V0 | 1 | V1 | 1
                for r in range(4):
                    def tunit(r=r):
                        jt = 4 * i + r
                        vtc = box[1]
                        tp = ps_small.tile([128, 128], BF16, tag="w",
                                           name=f"tp{jt}")
                        nc.tensor.transpose(tp, vtc[:, r * 128:(r + 1) * 128],
                                            ident_sb)
                        v_t = singles.tile([128, 130], BF16, tag=f"v{jt}",
                                           name=f"v{jt}")
                        nc.vector.tensor_copy(v_t[:, 0:64], tp[:, 0:64])
                        nc.vector.tensor_copy(v_t[:, 65:129], tp[:, 64:128])
                        nc.vector.memset(v_t[:, 64:65], 1.0)
                        nc.vector.memset(v_t[:, 129:130], 1.0)
                        v_tiles[jt] = v_t
                    yield tunit

        # up-front: only what step 0 needs (K0, V0, Q0-1); the rest drips
        # into the attention loop ahead of its first use.
        up_q = min(2, nch)
        for u_ in qk_chunk(kT, wk_sb, kt_tiles, "kt", 0):
            u_()
        for u_ in qk_chunk(vT, wv_sb, None, "vt", 0):
            u_()
        def q_chunk(i):
            return qk_chunk(qT, wq_sb, None, "qt", i,
                            pair_of=(qt_pairs, i // n_half, i % n_half))
        for i in range(up_q):
            for u_ in q_chunk(i):
                u_()
        drip = []
        for i in range(1, nch):
            drip.extend(qk_chunk(kT, wk_sb, kt_tiles, "kt", i))
            drip.extend(qk_chunk(vT, wv_sb, None, "vt", i))
        for i in range(up_q, nch):
            drip.extend(q_chunk(i))
        drip.reverse()  # pop() from the end

        # ---- deferred epilogue units ----
        def norm_unit(uraw, half, tag):
            def unit():
                sl = slice(half * 512, (half + 1) * 512)
                bc_ps = ps_small.tile([64, 512], F32, tag="w",
                                      name=f"bc{tag}_{half}")
                nc.tensor.matmul(bc_ps, lhsT=ones_sb[64:65, :],
                                 rhs=uraw[64:65, sl], start=True, stop=True)
                rbc = nrm_pool.tile([64, 512], F32, tag="rbc",
                                    name=f"rbc{tag}_{half}")
                nc.vector.reciprocal_approx_fast(out=rbc, in_=bc_ps)
                nc.vector.tensor_mul(uraw[0:64, sl], uraw[0:64, sl], rbc)
            return unit

        def out_unit(us, isup, it, wc, eng="v"):
            def unit():
                row0 = isup * is_chunk + it * 128
                po = ps_small.tile([128, 512], F32, tag="w",
                                   name=f"po{isup}_{it}_{wc}")
                nc.tensor.matmul(po, lhsT=us[0][0:64, it * 128:(it + 1) * 128],
                                 rhs=wo0_sb[:, wc * 512:(wc + 1) * 512],
                                 start=True, stop=False)
                nc.tensor.matmul(po, lhsT=us[1][0:64, it * 128:(it + 1) * 128],
                                 rhs=wo1_sb[:, wc * 512:(wc + 1) * 512],
                                 start=False, stop=True)
                ot = ostage.tile([128, 512], F32, tag="ot",
                                 name=f"ot{isup}_{it}_{wc}")
                if eng == "v":
                    nc.vector.tensor_copy(ot, po)
                else:
                    nc.scalar.copy(ot, po)
                dma_eng = nc.gpsimd if (it + wc) % 2 == 0 else nc.sync
                dma_eng.dma_start(
                    out=out[row0:row0 + 128, wc * 512:(wc + 1) * 512],
                    in_=ot)
            return unit

        # ---- software-pipelined attention (V matmuls 2 steps behind) ----
        steps = [(isup, h, jt)
                 for isup in range(nis) for h in range(2) for jt in range(njt)]
        pending_v = []   # up to 2 entries
        drains = []      # (due_step, fn): PSUM accumulator drain copies
        epi = []         # (due_step, fn): normalize + out-projection units
        us = None
        accs = {}

        def flush_v():
            acc_p, vslice_p, pt_p, last_p = pending_v.pop(0)
            for half in range(n_half):
                nc.tensor.matmul(
                    acc_p[:, half * 512:(half + 1) * 512],
                    lhsT=vslice_p,
                    rhs=pt_p[:, half * 512:(half + 1) * 512],
                    start=(last_p == 0), stop=(last_p == njt - 1))

        for t, (isup, h, jt) in enumerate(steps):
            if jt == 0:
                if h == 0:
                    us = [None, None]
                accs[(isup, h)] = ps_acc.tile([65, is_chunk], F32, tag="acc",
                                              name=f"acc{isup}_{h}")
            # scores + exp for step t
            sc = ps_scores.tile([128, is_chunk], F32, tag="s",
                                name=f"sc{isup}_{h}_{jt}")
            ktile = kt_tiles[jt // 4]
            for half in range(n_half):
                nc.tensor.matmul(
                    sc[:, half * 512:(half + 1) * 512],
                    lhsT=ktile[h * 64:(h + 1) * 64,
                               (jt % 4) * 128:(jt % 4) * 128 + 128],
                    rhs=qt_pairs[isup][h * 64:(h + 1) * 64,
                                       half * 512:(half + 1) * 512],
                    start=True, stop=True)
            pt = pt_pool.tile([128, is_chunk], BF16, tag="pt",
                              name=f"pt{isup}_{h}_{jt}")
            nc.scalar.activation(pt, sc, EXP, scale=0.125)
            # V matmuls of step t-3
            if len(pending_v) == 3:
                flush_v()
            acc = accs[(isup, h)]
            vlo = 0 if h == 0 else 65
            pending_v.append((acc, v_tiles[jt][:, vlo:vlo + 65], pt, jt))
            # projection drip after this step's scores so its DMA waits only
            # gate the NEXT step's exp
            for _ in range(6 if t < 32 else (4 if t < 48 else 2)):
                if drip:
                    drip.pop()()
            # deferred work whose inputs are long ready
            while drains and drains[0][0] <= t:
                drains.pop(0)[1]()
            if t % 2 == 0 and epi and epi[0][0] <= t:
                epi.pop(0)[1]()
            if jt == njt - 1:
                # head seam: drain the PSUM accumulator once its final V
                # matmuls (emitted at step t+3) are in; normalize later.
                uraw = u_pool.tile([65, is_chunk], BF16, tag=f"u{h}",
                                   name=f"uraw{isup}_{h}")
                us[h] = uraw
                def drain(acc=acc, uraw=uraw):
                    nc.vector.tensor_copy(uraw, acc)
                    # filler matmuls: keep the PE busy across the seam's DVE
                    # latency so HAM never clock-gates the array
                    for _ in range(6):
                        nc.tensor.matmul(junk, lhsT=ident_sb, rhs=ident_sb,
                                         start=True, stop=True)
                drains.append((t + 3, drain))
                if t + 1 < len(steps):
                    epi.extend((t + 4, norm_unit(uraw, half, f"{isup}_{h}"))
                               for half in range(n_half))
                else:
                    tail_norm = [norm_unit(uraw, half, f"{isup}_{h}")
                                 for half in range(n_half)]
                if h == 1 and isup < nis - 1:
                    epi.extend((t + 5, out_unit(us, isup, it, wc))
                               for it in range(is_chunk // 128)
                               for wc in range(DOUT // 512))

        # ---- tail: last head's V matmuls, drain, normalize + out-proj ----
        while pending_v:
            flush_v()
        while drip:
            drip.pop()()
        for _, fn in drains:
            fn()
        # keep the PE warm across the DVE drain/normalize latency
        for _ in range(14):
            nc.tensor.matmul(junk, lhsT=ident_sb, rhs=ident_sb,
                             start=True, stop=True)
        for _, fn in epi:
            fn()
        isup = nis - 1
        nh = is_chunk // 128
        for half in range(n_half):
            tail_norm[half]()
            for it in range(half * nh // n_half, (half + 1) * nh // n_half):
                for wc in range(DOUT // 512):
                    out_unit(us, isup, it, wc,
                             eng=("s" if (it + wc) % 2 else "v"))()


def build(n=N, din=DIN):
    nc = bacc.Bacc("TRN2", target_bir_lowering=False, debug=False,
                   num_devices=NCORES)
    qT = nc.dram_tensor("qT", [din, n], BF16, kind="ExternalInput").ap()
    kT = nc.dram_tensor("kT", [din, n], BF16, kind="ExternalInput").ap()
    vT = nc.dram_tensor("vT", [din, n], BF16, kind="ExternalInput").ap()
    wq = nc.dram_tensor("wq", [din, DPC], BF16, kind="ExternalInput").ap()
    wk = nc.dram_tensor("wk", [din, DPC], BF16, kind="ExternalInput").ap()
    wv = nc.dram_tensor("wv", [din, DPC], BF16, kind="ExternalInput").ap()
    wo = nc.dram_tensor("wo", [DPC, DOUT], BF16, kind="ExternalInput").ap()
    ident = nc.dram_tensor("ident", [128, 128], BF16, kind="ExternalInput").ap()
    out = nc.dram_tensor("out", [n, DOUT], F32, kind="ExternalOutput").ap()
    with tile.TileContext(nc) as tc:
        emit(tc, [out], [qT, kT, vT, wq, wk, wv, wo, ident], n, din)
    nc.compile()
    return nc


_NC_CACHE = {}


def _get_nc(n=N, din=DIN):
    key = (n, din)
    if key not in _NC_CACHE:
        _NC_CACHE[key] = build(n, din)
    return _NC_CACHE[key]


def make_in_maps(q, k, v, W_q, W_k, W_v, W_o):
    bf = ml_dtypes.bfloat16
    qT = np.ascontiguousarray(np.asarray(q, dtype=np.float32).T).astype(bf)
    kT = np.ascontiguousarray(np.asarray(k, dtype=np.float32).T).astype(bf)
    vT = np.ascontiguousarray(np.asarray(v, dtype=np.float32).T).astype(bf)
    W_q = np.asarray(W_q, dtype=np.float32)
    W_k = np.asarray(W_k, dtype=np.float32)
    W_v = np.asarray(W_v, dtype=np.float32)
    W_o = np.asarray(W_o, dtype=np.float32)
    ident = np.eye(128, dtype=bf)
    in_maps = []
    for c in range(NCORES):
        sl = slice(DPC * c, DPC * (c + 1))
        in_maps.append({
            "qT": qT, "kT": kT, "vT": vT,
            "wq": np.ascontiguousarray(W_q[:, sl]).astype(bf),
            "wk": np.ascontiguousarray(W_k[:, sl]).astype(bf),
            "wv": np.ascontiguousarray(W_v[:, sl]).astype(bf),
            "wo": np.ascontiguousarray(W_o[sl, :]).astype(bf),
            "ident": ident,
        })
    return in_maps


def run(q, k, v, W_q, W_k, W_v, W_o, trace=False):
    n = q.shape[0]
    nc = _get_nc(n=n, din=q.shape[1])
    in_maps = make_in_maps(q, k, v, W_q, W_k, W_v, W_o)
    res = run_bass_kernel_spmd(nc, in_maps, list(range(NCORES)), trace=trace)
    out = res.results[0]["out"].astype(np.float32)
    for c in range(1, NCORES):
        out += res.results[c]["out"]
    return out, res


def kernel(q, k, v, W_q, W_k, W_v, W_o):
    out, _ = run(q, k, v, W_q, W_k, W_v, W_o)
    return out
